# revision 16
# baseline (speedup 1.0000x reference)
"""Bidirectional CfC (AutoNCP-wired) Trainium2 Bass kernel.

Strategy
--------
The network is: encoder MLP (9->256->256->256, SiLU) -> two CfC scans
(fwd + bwd over T=2048, 3 stacked cells of sizes 20/12/32) -> comb MLP
(64->256->256) -> decoder MLP (256->128->64->2).

The MLP parts are embarrassingly parallel over B*T tokens and run
data-parallel over all 8 cores (sharded by time, transposed layout:
features on partitions, tokens on the free dim).  The CfC scans are
strictly sequential in T and latency-bound, so each direction runs on a
single core (core 0 fwd, core 1 bwd) with batch=32 on the free dim.

Cell math is rewritten in "sigmoid space": with tanh(x) = 2*sigmoid(2x)-1
and state u = (h+1)/2, every gate becomes a plain sigmoid and the affine
corrections fold into host-preprocessed weights/biases.  The three cells
run as a skewed pipeline (tick k computes h0(k), h1(k-1), h2(k-2)) so the
per-tick critical path is a single cell: matmul -> sigmoid -> 3 vector ops.

Three SPMD launches:
  A (8 cores): encoder + layer-0 input projections ("pre0", 60 gate rows)
  B (2 cores): the scans (same program, per-core data: fwd / time-flipped bwd)
  C (8 cores): comb + decoder on the scan outputs
"""

import os
import numpy as np
from contextlib import ExitStack

import concourse.bass as bass
import concourse.tile as tile
from concourse import bacc, mybir
from concourse.bass_utils import run_bass_kernel_spmd

FP = mybir.dt.float32
AF = mybir.ActivationFunctionType

B, T, IN, H = 32, 2048, 9, 256
N0, N1, N2 = 20, 12, 32          # CfC layer sizes (inter / cmd / motor)
NCORES = 8
TC = T // NCORES                 # timesteps per core in bulk launches
TOK = TC * B                     # tokens per core (t-major, b-minor)
CHUNK = 2048                     # token chunk in bulk launches
SCAN_CH = 128                    # timesteps per DMA chunk in the scan

_prog_cache = {}


# ----------------------------------------------------------------- launch A
def _build_launch_a():
    nc = bacc.Bacc("TRN2", target_bir_lowering=False, debug=False,
                   num_devices=NCORES)
    xT = nc.dram_tensor("xT", [IN, TOK], FP, kind="ExternalInput").ap()
    w1 = nc.dram_tensor("w1", [IN, H], FP, kind="ExternalInput").ap()
    b1 = nc.dram_tensor("b1", [H // 2, 2], FP, kind="ExternalInput").ap()
    w2 = nc.dram_tensor("w2", [H // 2, 2 * H], FP, kind="ExternalInput").ap()
    b2 = nc.dram_tensor("b2", [H // 2, 2], FP, kind="ExternalInput").ap()
    w3 = nc.dram_tensor("w3", [H // 2, 2 * H], FP, kind="ExternalInput").ap()
    pxf = nc.dram_tensor("pxf", [H // 2, 192], FP, kind="ExternalInput").ap()
    pbf = nc.dram_tensor("pbf", [96, 1], FP, kind="ExternalInput").ap()
    pxb = nc.dram_tensor("pxb", [H // 2, 192], FP, kind="ExternalInput").ap()
    pbb = nc.dram_tensor("pbb", [96, 1], FP, kind="ExternalInput").ap()
    of = nc.dram_tensor("pre0f", [32, TC * 96], FP, kind="ExternalOutput").ap()
    ob = nc.dram_tensor("pre0b", [32, TC * 96], FP, kind="ExternalOutput").ap()

    with tile.TileContext(nc) as tc:
        with ExitStack() as ctx:
            cpool = ctx.enter_context(tc.tile_pool(name="const", bufs=1))
            xpool = ctx.enter_context(tc.tile_pool(name="x", bufs=2))
            hpool = ctx.enter_context(tc.tile_pool(name="h", bufs=2))
            opool = ctx.enter_context(tc.tile_pool(name="o", bufs=2))
            pspool = ctx.enter_context(tc.tile_pool(name="ps", bufs=2,
                                                    space="PSUM"))

            # 256-row weights are stored in SBUF as (128, 2*cols):
            # w_sb[a, kh*cols + c] = w[kh*128 + a, c]
            w1s = cpool.tile([IN, H], FP, tag="w1")
            nc.sync.dma_start(w1s[:], w1[:])
            b1s = cpool.tile([H // 2, 2], FP, tag="b1")
            nc.sync.dma_start(b1s[:], b1[:])
            w2s = cpool.tile([H // 2, 2 * H], FP, tag="w2")
            nc.sync.dma_start(w2s[:], w2[:])
            b2s = cpool.tile([H // 2, 2], FP, tag="b2")
            nc.sync.dma_start(b2s[:], b2[:])
            w3s = cpool.tile([H // 2, 2 * H], FP, tag="w3")
            nc.sync.dma_start(w3s[:], w3[:])
            pxfs = cpool.tile([H // 2, 192], FP, tag="pxf")
            nc.sync.dma_start(pxfs[:], pxf[:])
            pbfs = cpool.tile([96, 1], FP, tag="pbf")
            nc.sync.dma_start(pbfs[:], pbf[:])
            pxbs = cpool.tile([H // 2, 192], FP, tag="pxb")
            nc.sync.dma_start(pxbs[:], pxb[:])
            pbbs = cpool.tile([96, 1], FP, tag="pbb")
            nc.sync.dma_start(pbbs[:], pbb[:])

            nt = CHUNK // 512
            for c0 in range(0, TOK, CHUNK):
                xs = xpool.tile([IN, CHUNK], FP, tag="xs")
                nc.sync.dma_start(xs[:], xT[:, c0:c0 + CHUNK])

                # ---- enc layer 1 (K=9) + SiLU
                h1 = [hpool.tile([128, CHUNK], FP, tag=f"h1_{i}", name=f"h1_{i}_{c0}") for i in range(2)]
                for half in range(2):
                    for j in range(nt):
                        ps = pspool.tile([128, 512], FP, tag="ps")
                        nc.tensor.matmul(ps[:], w1s[:, half * 128:(half + 1) * 128],
                                         xs[:, j * 512:(j + 1) * 512],
                                         start=True, stop=True)
                        nc.scalar.activation(h1[half][:, j * 512:(j + 1) * 512],
                                             ps[:], AF.Silu,
                                             bias=b1s[:, half:half + 1])
                # ---- enc layer 2 + SiLU
                h2 = [hpool.tile([128, CHUNK], FP, tag=f"h2_{i}", name=f"h2_{i}_{c0}") for i in range(2)]
                for half in range(2):
                    for j in range(nt):
                        ps = pspool.tile([128, 512], FP, tag="ps")
                        for kh in range(2):
                            nc.tensor.matmul(
                                ps[:],
                                w2s[:, kh * H + half * 128:kh * H + (half + 1) * 128],
                                h1[kh][:, j * 512:(j + 1) * 512],
                                start=(kh == 0), stop=(kh == 1))
                        nc.scalar.activation(h2[half][:, j * 512:(j + 1) * 512],
                                             ps[:], AF.Silu,
                                             bias=b2s[:, half:half + 1])
                # ---- enc layer 3 (no activation; bias folded into pre0)
                h3 = [hpool.tile([128, CHUNK], FP, tag=f"h3_{i}", name=f"h3_{i}_{c0}") for i in range(2)]
                for half in range(2):
                    for j in range(nt):
                        ps = pspool.tile([128, 512], FP, tag="ps")
                        for kh in range(2):
                            nc.tensor.matmul(
                                ps[:],
                                w3s[:, kh * H + half * 128:kh * H + (half + 1) * 128],
                                h2[kh][:, j * 512:(j + 1) * 512],
                                start=(kh == 0), stop=(kh == 1))
                        nc.scalar.copy(h3[half][:, j * 512:(j + 1) * 512], ps[:])
                # ---- pre0 projections (fwd & bwd)
                # po rows: [s1(20)+b1row(12) | s2(20)+b1row(12) | g(20)+b1row(12)]
                # DRAM pre0 layout: (32, t*96): col = t*96 + gate*32 + b
                for pxs, pbs, out in ((pxfs, pbfs, of), (pxbs, pbbs, ob)):
                    po = opool.tile([96, CHUNK], FP, tag="po")
                    for j in range(nt):
                        ps = pspool.tile([96, 512], FP, tag="ps96")
                        for kh in range(2):
                            nc.tensor.matmul(
                                ps[:],
                                pxs[:, kh * 96:(kh + 1) * 96],
                                h3[kh][:, j * 512:(j + 1) * 512],
                                start=(kh == 0), stop=(kh == 1))
                        nc.vector.tensor_scalar_add(
                            po[:, j * 512:(j + 1) * 512], ps[:], pbs[:])
                    t0, t1 = c0 // B, (c0 + CHUNK) // B
                    dst = out.rearrange("r (t x) -> r t x", x=96)
                    for gi in range(3):
                        nc.sync.dma_start(
                            dst[:, t0:t1, gi * 32:(gi + 1) * 32],
                            po[gi * 32:(gi + 1) * 32, :].rearrange(
                                "r (t b) -> r t b", b=B))
    nc.compile()
    return nc


# ----------------------------------------------------------------- launch B
def _build_launch_b(t_steps=T, scan_ch=SCAN_CH):
    nc = bacc.Bacc("TRN2", target_bir_lowering=False, debug=False,
                   num_devices=2)
    pre0 = nc.dram_tensor("pre0", [32, t_steps * 96], FP,
                          kind="ExternalInput").ap()
    wts = nc.dram_tensor("wts", [128, 96], FP, kind="ExternalInput").ap()
    bias2 = nc.dram_tensor("bias2", [32, 96], FP, kind="ExternalInput").ap()
    iden = nc.dram_tensor("ident", [32, 32], FP, kind="ExternalInput").ap()
    yout = nc.dram_tensor("y", [N2, t_steps * B], FP,
                          kind="ExternalOutput").ap()

    with tile.TileContext(nc) as tc:
        with ExitStack() as ctx:
            cpool = ctx.enter_context(tc.tile_pool(name="const", bufs=1))
            ppool = ctx.enter_context(tc.tile_pool(name="pre0", bufs=2))
            ypool = ctx.enter_context(tc.tile_pool(name="y", bufs=2))
            upool = ctx.enter_context(tc.tile_pool(name="u", bufs=3))
            spool = ctx.enter_context(tc.tile_pool(name="s", bufs=3))
            psa = ctx.enter_context(tc.tile_pool(name="psa", bufs=2,
                                                 space="PSUM"))
            psb = ctx.enter_context(tc.tile_pool(name="psb", bufs=2,
                                                 space="PSUM"))

            ws = cpool.tile([128, 96], FP, tag="w")
            nc.sync.dma_start(ws[:], wts[:])
            b2s = cpool.tile([32, 96], FP, tag="b2")
            nc.sync.dma_start(b2s[:], bias2[:])
            idn = cpool.tile([32, 32], FP, tag="id")
            nc.sync.dma_start(idn[:], iden[:])

            u_prev = upool.tile([128, 32], FP, tag="u")
            nc.vector.memset(u_prev[:], 0.5)

            pre_tiles = [None] * ((t_steps + scan_ch - 1) // scan_ch)
            y_tile = None
            for k in range(t_steps + 2):
                if k < t_steps and k % scan_ch == 0:
                    ci = k // scan_ch
                    w = min(scan_ch, t_steps - k) * 96
                    pt = ppool.tile([32, scan_ch * 96], FP, tag="p")
                    nc.sync.dma_start(pt[:, 0:w], pre0[:, k * 96:k * 96 + w])
                    pre_tiles[ci] = pt
                col = min(k, t_steps - 1)
                pt = pre_tiles[col // scan_ch]
                cc = (col % scan_ch) * 96

                pa = psa.tile([64, 64], FP, tag="pa")
                pb = psb.tile([64, 32], FP, tag="pb")
                # pre0 / bias injection (identity matmuls; I is full 32x32)
                nc.tensor.matmul(pa[0:32, :], idn[:], pt[:, cc:cc + 64],
                                 start=True, stop=False, tile_position=(0, 0))
                nc.tensor.matmul(pa[32:64, :], idn[:], b2s[:, 0:64],
                                 start=True, stop=False, tile_position=(0, 32))
                nc.tensor.matmul(pb[0:32, :], idn[:], pt[:, cc + 64:cc + 96],
                                 start=True, stop=False, tile_position=(0, 0))
                nc.tensor.matmul(pb[32:64, :], idn[:], b2s[:, 64:96],
                                 start=True, stop=False, tile_position=(0, 32))
                # recurrent matmuls: cells 0+1 (merged, K=32) and cell 2 (K=44)
                nc.tensor.matmul(pa[0:32, 0:32], ws[0:32, 0:32], u_prev[0:32, :],
                                 start=False, stop=False, tile_position=(0, 0))
                nc.tensor.matmul(pa[0:32, 32:64], ws[0:32, 32:64],
                                 u_prev[0:32, :], start=False, stop=True,
                                 tile_position=(0, 0))
                nc.tensor.matmul(pa[32:64, 0:32], ws[64:128, 0:32],
                                 u_prev[64:128, :], start=False, stop=False,
                                 tile_position=(64, 32))
                nc.tensor.matmul(pa[32:64, 32:64], ws[64:128, 32:64],
                                 u_prev[64:128, :], start=False, stop=True,
                                 tile_position=(64, 32))
                nc.tensor.matmul(pb[0:32, :], ws[0:32, 64:96], u_prev[0:32, :],
                                 start=False, stop=True, tile_position=(0, 0))
                nc.tensor.matmul(pb[32:64, :], ws[64:128, 64:96],
                                 u_prev[64:128, :], start=False, stop=True,
                                 tile_position=(64, 32))

                sa = spool.tile([64, 64], FP, tag="sa")
                nc.scalar.activation(sa[:], pa[:], AF.Sigmoid)
                g = spool.tile([64, 32], FP, tag="g")
                nc.scalar.activation(g[:], pb[:], AF.Sigmoid)

                d = spool.tile([64, 32], FP, tag="d")
                nc.gpsimd.tensor_sub(d[:], sa[:, 32:64], sa[:, 0:32])
                e = spool.tile([64, 32], FP, tag="e")
                nc.vector.tensor_mul(e[:], g[:], d[:])

                u_new = upool.tile([128, 32], FP, tag="u")
                # U rows: [u1(0:12); u0(12:32); u2(32:64);
                #          copy of rows 0:32 at 64:96; u2 copy at 96:128]
                nc.vector.tensor_add(u_new[0:64, :], sa[:, 0:32], e[:])
                nc.vector.tensor_add(u_new[64:96, :], sa[0:32, 0:32],
                                     e[0:32, :])
                nc.vector.tensor_add(u_new[96:128, :], sa[32:64, 0:32],
                                     e[32:64, :])
                if k == 0:
                    nc.vector.memset(u_new[0:12, :], 0.5)
                    nc.vector.memset(u_new[32:64, :], 0.5)
                    nc.vector.memset(u_new[64:96, :], 0.5)
                    nc.vector.memset(u_new[96:128, :], 0.5)
                elif k == 1:
                    nc.vector.memset(u_new[32:64, :], 0.5)
                    nc.vector.memset(u_new[96:128, :], 0.5)
                if k >= 2:
                    t_out = k - 2
                    if t_out % scan_ch == 0:
                        y_tile = ypool.tile([N2, scan_ch * B], FP, tag="y")
                    yc = (t_out % scan_ch) * B
                    nc.gpsimd.tensor_add(y_tile[:, yc:yc + B],
                                         sa[32:64, 0:32], e[32:64, :])
                    if t_out % scan_ch == scan_ch - 1 or t_out == t_steps - 1:
                        c0 = (t_out // scan_ch) * scan_ch * B
                        nc.sync.dma_start(
                            yout[:, c0:c0 + (t_out % scan_ch + 1) * B],
                            y_tile[:, 0:(t_out % scan_ch + 1) * B])
                u_prev = u_new
    nc.compile()
    return nc


# ----------------------------------------------------------------- launch C
def _build_launch_c():
    nc = bacc.Bacc("TRN2", target_bir_lowering=False, debug=False,
                   num_devices=NCORES)
    yf = nc.dram_tensor("yf", [N2, TOK], FP, kind="ExternalInput").ap()
    yb = nc.dram_tensor("yb", [N2, TOK], FP, kind="ExternalInput").ap()
    wc1 = nc.dram_tensor("wc1", [2 * N2, H], FP, kind="ExternalInput").ap()
    bc1 = nc.dram_tensor("bc1", [H // 2, 2], FP, kind="ExternalInput").ap()
    wc2 = nc.dram_tensor("wc2", [H // 2, 2 * H], FP, kind="ExternalInput").ap()
    wd1 = nc.dram_tensor("wd1", [H // 2, 256], FP, kind="ExternalInput").ap()
    bd1 = nc.dram_tensor("bd1", [128, 1], FP, kind="ExternalInput").ap()
    wd2 = nc.dram_tensor("wd2", [128, 64], FP, kind="ExternalInput").ap()
    bd2 = nc.dram_tensor("bd2", [64, 1], FP, kind="ExternalInput").ap()
    wd3 = nc.dram_tensor("wd3", [64, 2], FP, kind="ExternalInput").ap()
    bd3 = nc.dram_tensor("bd3", [2, 1], FP, kind="ExternalInput").ap()
    psi = nc.dram_tensor("psi", [2, TOK], FP, kind="ExternalOutput").ap()

    with tile.TileContext(nc) as tc:
        with ExitStack() as ctx:
            cpool = ctx.enter_context(tc.tile_pool(name="const", bufs=1))
            upool = ctx.enter_context(tc.tile_pool(name="u", bufs=2))
            hpool = ctx.enter_context(tc.tile_pool(name="h", bufs=2))
            opool = ctx.enter_context(tc.tile_pool(name="o", bufs=2))
            pspool = ctx.enter_context(tc.tile_pool(name="ps", bufs=2,
                                                    space="PSUM"))

            wc1s = cpool.tile([2 * N2, H], FP, tag="wc1")
            nc.sync.dma_start(wc1s[:], wc1[:])
            bc1s = cpool.tile([H // 2, 2], FP, tag="bc1")
            nc.sync.dma_start(bc1s[:], bc1[:])
            wc2s = cpool.tile([H // 2, 2 * H], FP, tag="wc2")
            nc.sync.dma_start(wc2s[:], wc2[:])
            wd1s = cpool.tile([H // 2, 256], FP, tag="wd1")
            nc.sync.dma_start(wd1s[:], wd1[:])
            bd1s = cpool.tile([128, 1], FP, tag="bd1")
            nc.sync.dma_start(bd1s[:], bd1[:])
            wd2s = cpool.tile([128, 64], FP, tag="wd2")
            nc.sync.dma_start(wd2s[:], wd2[:])
            bd2s = cpool.tile([64, 1], FP, tag="bd2")
            nc.sync.dma_start(bd2s[:], bd2[:])
            wd3s = cpool.tile([64, 2], FP, tag="wd3")
            nc.sync.dma_start(wd3s[:], wd3[:])
            bd3s = cpool.tile([2, 1], FP, tag="bd3")
            nc.sync.dma_start(bd3s[:], bd3[:])

            nt = CHUNK // 512
            for c0 in range(0, TOK, CHUNK):
                us = upool.tile([2 * N2, CHUNK], FP, tag="us")
                nc.sync.dma_start(us[0:N2, :], yf[:, c0:c0 + CHUNK])
                nc.sync.dma_start(us[N2:2 * N2, :], yb[:, c0:c0 + CHUNK])
                # comb layer 1 (K=64) + SiLU
                c1 = [hpool.tile([128, CHUNK], FP, tag=f"c1_{i}", name=f"c1_{i}_{c0}") for i in range(2)]
                for half in range(2):
                    for j in range(nt):
                        ps = pspool.tile([128, 512], FP, tag="ps")
                        nc.tensor.matmul(ps[:],
                                         wc1s[:, half * 128:(half + 1) * 128],
                                         us[:, j * 512:(j + 1) * 512],
                                         start=True, stop=True)
                        nc.scalar.activation(c1[half][:, j * 512:(j + 1) * 512],
                                             ps[:], AF.Silu,
                                             bias=bc1s[:, half:half + 1])
                # comb layer 2 (no act, bias folded into dec1)
                c2 = [hpool.tile([128, CHUNK], FP, tag=f"c2_{i}", name=f"c2_{i}_{c0}") for i in range(2)]
                for half in range(2):
                    for j in range(nt):
                        ps = pspool.tile([128, 512], FP, tag="ps")
                        for kh in range(2):
                            nc.tensor.matmul(
                                ps[:],
                                wc2s[:, kh * H + half * 128:kh * H + (half + 1) * 128],
                                c1[kh][:, j * 512:(j + 1) * 512],
                                start=(kh == 0), stop=(kh == 1))
                        nc.scalar.copy(c2[half][:, j * 512:(j + 1) * 512], ps[:])
                # dec layer 1 (256->128) + SiLU
                d1 = hpool.tile([128, CHUNK], FP, tag="d1")
                for j in range(nt):
                    ps = pspool.tile([128, 512], FP, tag="ps")
                    for kh in range(2):
                        nc.tensor.matmul(ps[:], wd1s[:, kh * 128:(kh + 1) * 128],
                                         c2[kh][:, j * 512:(j + 1) * 512],
                                         start=(kh == 0), stop=(kh == 1))
                    nc.scalar.activation(d1[:, j * 512:(j + 1) * 512], ps[:],
                                         AF.Silu, bias=bd1s[:])
                # dec layer 2 (128->64) + SiLU
                d2 = hpool.tile([64, CHUNK], FP, tag="d2")
                for j in range(nt):
                    ps = pspool.tile([64, 512], FP, tag="ps64")
                    nc.tensor.matmul(ps[:], wd2s[:], d1[:, j * 512:(j + 1) * 512],
                                     start=True, stop=True)
                    nc.scalar.activation(d2[:, j * 512:(j + 1) * 512], ps[:],
                                         AF.Silu, bias=bd2s[:])
                # dec layer 3 (64->2) + bias
                po = opool.tile([2, CHUNK], FP, tag="po")
                for j in range(nt):
                    ps = pspool.tile([2, 512], FP, tag="ps2")
                    nc.tensor.matmul(ps[:], wd3s[:], d2[:, j * 512:(j + 1) * 512],
                                     start=True, stop=True)
                    nc.vector.tensor_scalar_add(po[:, j * 512:(j + 1) * 512],
                                                ps[:], bd3s[:])
                nc.sync.dma_start(psi[:, c0:c0 + CHUNK], po[:])
    nc.compile()
    return nc


# ------------------------------------------------------------- host helpers
def _np(a):
    return np.asarray(a, dtype=np.float32)


def _pack2(w):
    """(256, C) -> (128, 2C): halves of the contraction dim side by side."""
    return np.ascontiguousarray(np.concatenate([w[:128], w[128:]], axis=1))


def _gate_parts(p, m, d):
    """Masked x/h weight parts + bias for each gate of one CfC cell."""
    out = {}
    m = _np(m)
    for gname in ("ff1", "ff2", "ta", "tb"):
        wm = _np(p[gname + "_w"]) * m
        out[gname] = (wm[:d], wm[d:], _np(p[gname + "_b"]))
    return out


def _scan_params(cfc, masks, b3):
    """Host-side weight transforms for one scan direction.

    sigmoid-space: state u = (h+1)/2, gates s = sigma(...).  A weight W
    consuming a (2u-1) input becomes 2W with bias correction -colsum(W);
    ff gates additionally get an overall factor 2 (tanh(x)=2*sig(2x)-1).

    Returns:
      px   (H, 96): pre0 projection weights.  Unit order inside each
                    32-row gate block is [L1-bias rows (12) | L0 units (20)]
      pb   (96, 1): pre0 bias; rows c:c+12 carry the L1 gate biases
      wts  (128, 96): recurrent weights; input rows [u1(12); u0(20)] at
                    0:32 and [u1copy(12); u0copy(20, zero); u2(32)] at 64:128
      bias2 (32, 96): L2 gate biases broadcast over batch
    """
    g0 = _gate_parts(cfc[0], masks[0], H)
    g1 = _gate_parts(cfc[1], masks[1], N0)
    g2 = _gate_parts(cfc[2], masks[2], N1)

    px = np.zeros((H, 96), np.float32)
    pb = np.zeros((96, 1), np.float32)
    wts = np.zeros((128, 96), np.float32)
    bias2 = np.zeros((32, 96), np.float32)
    for i, gn in enumerate(("ff1", "ff2", "g")):
        if gn == "g":
            w0x = g0["ta"][0] + g0["tb"][0]
            w0h = g0["ta"][1] + g0["tb"][1]
            b0v = g0["ta"][2] + g0["tb"][2]
            w1x = g1["ta"][0] + g1["tb"][0]
            w1h = g1["ta"][1] + g1["tb"][1]
            b1v = g1["ta"][2] + g1["tb"][2]
            w2x = g2["ta"][0] + g2["tb"][0]
            w2h = g2["ta"][1] + g2["tb"][1]
            b2v = g2["ta"][2] + g2["tb"][2]
            f = 1.0
        else:
            w0x, w0h, b0v = g0[gn]
            w1x, w1h, b1v = g1[gn]
            w2x, w2h, b2v = g2[gn]
            f = 2.0
        c = i * 32
        px[:, c + 12:c + 32] = f * w0x
        pb[c:c + 12, 0] = f * (b1v - w1x.sum(0) - w1h.sum(0))
        pb[c + 12:c + 32, 0] = f * (b0v + b3 @ w0x - w0h.sum(0))
        # sa unit order per gate block: [L1 units (12) | L0 units (20)]
        wts[0:12, c:c + 12] = 2.0 * f * w1h          # u1 -> L1
        wts[12:32, c:c + 12] = 2.0 * f * w1x         # u0 -> L1
        wts[12:32, c + 12:c + 32] = 2.0 * f * w0h    # u0 -> L0
        wts[64:76, c:c + 32] = 2.0 * f * w2x         # u1 copy -> L2
        wts[96:128, c:c + 32] = 2.0 * f * w2h        # u2 copy -> L2
        bias2[:, c:c + 32] = (f * (b2v - w2x.sum(0) - w2h.sum(0)))[:, None]
    return px, pb, wts, bias2


def _ident_tile():
    return np.eye(32, dtype=np.float32)


# ------------------------------------------------------------------- kernel
def kernel(x, enc, cfc_f, cfc_b, comb, dec, masks_f, masks_b):
    if "A" not in _prog_cache:
        _prog_cache["A"] = _build_launch_a()
        _prog_cache["B"] = _build_launch_b()
        _prog_cache["C"] = _build_launch_c()
    ncA, ncB, ncC = _prog_cache["A"], _prog_cache["B"], _prog_cache["C"]

    x = _np(x)
    (w1, b1), (w2, b2), (w3, b3) = [(_np(w), _np(b)) for w, b in enc]
    pxf, pbf, wtsf, bias2f = _scan_params(cfc_f, masks_f, b3)
    pxb, pbb, wtsb, bias2b = _scan_params(cfc_b, masks_b, b3)
    iden = _ident_tile()

    # ---- launch A: encoder + pre0, time-sharded across 8 cores
    xT = np.ascontiguousarray(x.transpose(2, 1, 0).reshape(IN, T * B))
    common_a = dict(w1=w1, b1=_pack2(b1.reshape(H, 1)), w2=_pack2(w2),
                    b2=_pack2(b2.reshape(H, 1)), w3=_pack2(w3),
                    pxf=_pack2(pxf), pbf=pbf, pxb=_pack2(pxb), pbb=pbb)
    in_maps = [dict(common_a, xT=np.ascontiguousarray(
        xT[:, c * TOK:(c + 1) * TOK])) for c in range(NCORES)]
    res_a = run_bass_kernel_spmd(ncA, in_maps, core_ids=list(range(NCORES)))
    pre0f = np.concatenate([res_a.results[c]["pre0f"] for c in range(NCORES)],
                           axis=1)
    pre0b = np.concatenate([res_a.results[c]["pre0b"] for c in range(NCORES)],
                           axis=1)
    # bwd scan consumes time-reversed sequence
    pre0b = np.ascontiguousarray(
        pre0b.reshape(32, T, 96)[:, ::-1, :].reshape(32, T * 96))

    # ---- launch B: the two scans (core 0 fwd, core 1 bwd)
    in_b = [dict(pre0=pre0f, wts=wtsf, bias2=bias2f, ident=iden),
            dict(pre0=pre0b, wts=wtsb, bias2=bias2b, ident=iden)]
    res_b = run_bass_kernel_spmd(ncB, in_b, core_ids=[0, 1])
    y_f = res_b.results[0]["y"]
    y_b = res_b.results[1]["y"]
    y_b = np.ascontiguousarray(
        y_b.reshape(N2, T, B)[:, ::-1, :].reshape(N2, T * B))

    # ---- launch C: comb + dec
    (wc1, bc1), (wc2, bc2) = [(_np(w), _np(b)) for w, b in comb]
    (wd1, bd1), (wd2, bd2), (wd3, bd3) = [(_np(w), _np(b)) for w, b in dec]
    wc1t = 2.0 * wc1
    bc1t = bc1 - wc1.sum(0)
    bd1t = bd1 + bc2 @ wd1
    common_c = dict(wc1=wc1t, bc1=_pack2(bc1t.reshape(H, 1)), wc2=_pack2(wc2),
                    wd1=_pack2(wd1), bd1=bd1t.reshape(128, 1),
                    wd2=wd2, bd2=bd2.reshape(64, 1),
                    wd3=wd3, bd3=bd3.reshape(2, 1))
    in_maps = [dict(common_c,
                    yf=np.ascontiguousarray(y_f[:, c * TOK:(c + 1) * TOK]),
                    yb=np.ascontiguousarray(y_b[:, c * TOK:(c + 1) * TOK]))
               for c in range(NCORES)]
    res_c = run_bass_kernel_spmd(ncC, in_maps, core_ids=list(range(NCORES)))
    psi = np.concatenate([res_c.results[c]["psi"] for c in range(NCORES)],
                         axis=1)                     # (2, T*B) t-major
    psi = psi.reshape(2, T, B).transpose(0, 2, 1)    # (2, B, T)
    return np.ascontiguousarray(psi)


# revision 24
# speedup vs baseline: 1.1893x; 1.1893x over previous
"""Bidirectional CfC (AutoNCP-wired) Trainium2 Bass kernel.

Strategy
--------
The network is: encoder MLP (9->256->256->256, SiLU) -> two CfC scans
(fwd + bwd over T=2048, 3 stacked cells of sizes 20/12/32) -> comb MLP
(64->256->256) -> decoder MLP (256->128->64->2).

The MLP parts are embarrassingly parallel over B*T tokens and run
data-parallel over all 8 cores (sharded by time, transposed layout:
features on partitions, tokens on the free dim).  The CfC scans are
strictly sequential in T and latency-bound, so each direction runs on a
single core (core 0 fwd, core 1 bwd) with batch=32 on the free dim.

Cell math is rewritten in "sigmoid space": with tanh(x) = 2*sigmoid(2x)-1
and state u = (h+1)/2, every gate becomes a plain sigmoid and the affine
corrections fold into host-preprocessed weights/biases.  The three cells
run as a skewed pipeline (tick k computes h0(k), h1(k-1), h2(k-2)) so the
per-tick critical path is a single cell: matmul -> sigmoid -> 3 vector ops.

Three SPMD launches:
  A (8 cores): encoder + layer-0 input projections ("pre0", 60 gate rows)
  B (2 cores): the scans (same program, per-core data: fwd / time-flipped bwd)
  C (8 cores): comb + decoder on the scan outputs
"""

import os
import numpy as np
from contextlib import ExitStack

import concourse.bass as bass
import concourse.tile as tile
from concourse import bacc, mybir
from concourse.bass_utils import run_bass_kernel_spmd

FP = mybir.dt.float32
AF = mybir.ActivationFunctionType

B, T, IN, H = 32, 2048, 9, 256
N0, N1, N2 = 20, 12, 32          # CfC layer sizes (inter / cmd / motor)
NCORES = 8
TC = T // NCORES                 # timesteps per core in bulk launches
TOK = TC * B                     # tokens per core (t-major, b-minor)
CHUNK = 2048                     # token chunk in bulk launches
SCAN_CH = 128                    # timesteps per DMA chunk in the scan

_prog_cache = {}


# ----------------------------------------------------------------- launch A
def _build_launch_a():
    nc = bacc.Bacc("TRN2", target_bir_lowering=False, debug=False,
                   num_devices=NCORES)
    xT = nc.dram_tensor("xT", [IN, TOK], FP, kind="ExternalInput").ap()
    w1 = nc.dram_tensor("w1", [IN, H], FP, kind="ExternalInput").ap()
    b1 = nc.dram_tensor("b1", [H // 2, 2], FP, kind="ExternalInput").ap()
    w2 = nc.dram_tensor("w2", [H // 2, 2 * H], FP, kind="ExternalInput").ap()
    b2 = nc.dram_tensor("b2", [H // 2, 2], FP, kind="ExternalInput").ap()
    w3 = nc.dram_tensor("w3", [H // 2, 2 * H], FP, kind="ExternalInput").ap()
    pxf = nc.dram_tensor("pxf", [H // 2, 256], FP, kind="ExternalInput").ap()
    pbf = nc.dram_tensor("pbf", [128, 1], FP, kind="ExternalInput").ap()
    pxb = nc.dram_tensor("pxb", [H // 2, 256], FP, kind="ExternalInput").ap()
    pbb = nc.dram_tensor("pbb", [128, 1], FP, kind="ExternalInput").ap()
    of = nc.dram_tensor("pre0f", [32, TC * 128], FP, kind="ExternalOutput").ap()
    ob = nc.dram_tensor("pre0b", [32, TC * 128], FP, kind="ExternalOutput").ap()

    with tile.TileContext(nc) as tc:
        with ExitStack() as ctx:
            cpool = ctx.enter_context(tc.tile_pool(name="const", bufs=1))
            xpool = ctx.enter_context(tc.tile_pool(name="x", bufs=2))
            hpool = ctx.enter_context(tc.tile_pool(name="h", bufs=2))
            opool = ctx.enter_context(tc.tile_pool(name="o", bufs=2))
            pspool = ctx.enter_context(tc.tile_pool(name="ps", bufs=2,
                                                    space="PSUM"))

            # 256-row weights are stored in SBUF as (128, 2*cols):
            # w_sb[a, kh*cols + c] = w[kh*128 + a, c]
            w1s = cpool.tile([IN, H], FP, tag="w1")
            nc.sync.dma_start(w1s[:], w1[:])
            b1s = cpool.tile([H // 2, 2], FP, tag="b1")
            nc.sync.dma_start(b1s[:], b1[:])
            w2s = cpool.tile([H // 2, 2 * H], FP, tag="w2")
            nc.sync.dma_start(w2s[:], w2[:])
            b2s = cpool.tile([H // 2, 2], FP, tag="b2")
            nc.sync.dma_start(b2s[:], b2[:])
            w3s = cpool.tile([H // 2, 2 * H], FP, tag="w3")
            nc.sync.dma_start(w3s[:], w3[:])
            pxfs = cpool.tile([H // 2, 256], FP, tag="pxf")
            nc.sync.dma_start(pxfs[:], pxf[:])
            pbfs = cpool.tile([128, 1], FP, tag="pbf")
            nc.sync.dma_start(pbfs[:], pbf[:])
            pxbs = cpool.tile([H // 2, 256], FP, tag="pxb")
            nc.sync.dma_start(pxbs[:], pxb[:])
            pbbs = cpool.tile([128, 1], FP, tag="pbb")
            nc.sync.dma_start(pbbs[:], pbb[:])

            nt = CHUNK // 512
            for c0 in range(0, TOK, CHUNK):
                xs = xpool.tile([IN, CHUNK], FP, tag="xs")
                nc.sync.dma_start(xs[:], xT[:, c0:c0 + CHUNK])

                # ---- enc layer 1 (K=9) + SiLU
                h1 = [hpool.tile([128, CHUNK], FP, tag=f"h1_{i}", name=f"h1_{i}_{c0}") for i in range(2)]
                for half in range(2):
                    for j in range(nt):
                        ps = pspool.tile([128, 512], FP, tag="ps")
                        nc.tensor.matmul(ps[:], w1s[:, half * 128:(half + 1) * 128],
                                         xs[:, j * 512:(j + 1) * 512],
                                         start=True, stop=True)
                        nc.scalar.activation(h1[half][:, j * 512:(j + 1) * 512],
                                             ps[:], AF.Silu,
                                             bias=b1s[:, half:half + 1])
                # ---- enc layer 2 + SiLU
                h2 = [hpool.tile([128, CHUNK], FP, tag=f"h2_{i}", name=f"h2_{i}_{c0}") for i in range(2)]
                for half in range(2):
                    for j in range(nt):
                        ps = pspool.tile([128, 512], FP, tag="ps")
                        for kh in range(2):
                            nc.tensor.matmul(
                                ps[:],
                                w2s[:, kh * H + half * 128:kh * H + (half + 1) * 128],
                                h1[kh][:, j * 512:(j + 1) * 512],
                                start=(kh == 0), stop=(kh == 1))
                        nc.scalar.activation(h2[half][:, j * 512:(j + 1) * 512],
                                             ps[:], AF.Silu,
                                             bias=b2s[:, half:half + 1])
                # ---- enc layer 3 (no activation; bias folded into pre0)
                h3 = [hpool.tile([128, CHUNK], FP, tag=f"h3_{i}", name=f"h3_{i}_{c0}") for i in range(2)]
                for half in range(2):
                    for j in range(nt):
                        ps = pspool.tile([128, 512], FP, tag="ps")
                        for kh in range(2):
                            nc.tensor.matmul(
                                ps[:],
                                w3s[:, kh * H + half * 128:kh * H + (half + 1) * 128],
                                h2[kh][:, j * 512:(j + 1) * 512],
                                start=(kh == 0), stop=(kh == 1))
                        nc.scalar.copy(h3[half][:, j * 512:(j + 1) * 512], ps[:])
                # ---- pre0 projections (fwd & bwd)
                # po rows: [s1(20)+b1row(12) | s2(20)+b1row(12) | g(20)+b1row(12)]
                # DRAM pre0 layout: (32, t*96): col = t*96 + gate*32 + b
                for pxs, pbs, out in ((pxfs, pbfs, of), (pxbs, pbbs, ob)):
                    po = opool.tile([128, CHUNK], FP, tag="po")
                    for j in range(nt):
                        ps = pspool.tile([128, 512], FP, tag="ps96")
                        for kh in range(2):
                            nc.tensor.matmul(
                                ps[:],
                                pxs[:, kh * 128:(kh + 1) * 128],
                                h3[kh][:, j * 512:(j + 1) * 512],
                                start=(kh == 0), stop=(kh == 1))
                        nc.vector.tensor_scalar_add(
                            po[:, j * 512:(j + 1) * 512], ps[:], pbs[:])
                    t0, t1 = c0 // B, (c0 + CHUNK) // B
                    dst = out.rearrange("r (t x) -> r t x", x=128)
                    for gi in range(4):
                        nc.sync.dma_start(
                            dst[:, t0:t1, gi * 32:(gi + 1) * 32],
                            po[gi * 32:(gi + 1) * 32, :].rearrange(
                                "r (t b) -> r t b", b=B))
    nc.compile()
    return nc


# ----------------------------------------------------------------- launch B
def _build_launch_b(t_steps=T, scan_ch=SCAN_CH, repeats=1):
    nc = bacc.Bacc("TRN2", target_bir_lowering=False, debug=False,
                   num_devices=2)
    pre0 = nc.dram_tensor("pre0", [32, t_steps * 128], FP,
                          kind="ExternalInput").ap()
    wts = nc.dram_tensor("wts", [128, 256], FP, kind="ExternalInput").ap()
    ysel = nc.dram_tensor("ysel", [128, 32], FP, kind="ExternalInput").ap()
    bias2 = nc.dram_tensor("bias2", [32, 128], FP, kind="ExternalInput").ap()
    iden = nc.dram_tensor("ident", [32, 32], FP, kind="ExternalInput").ap()
    yout = nc.dram_tensor("y", [N2, t_steps * B], FP,
                          kind="ExternalOutput").ap()

    with tile.TileContext(nc) as tc:
        with ExitStack() as ctx:
            cpool = ctx.enter_context(tc.tile_pool(name="const", bufs=1))
            ppool = ctx.enter_context(tc.tile_pool(name="pre0", bufs=2))
            ypool = ctx.enter_context(tc.tile_pool(name="y", bufs=2))
            upool = ctx.enter_context(tc.tile_pool(name="u", bufs=3))
            spool = ctx.enter_context(tc.tile_pool(name="s", bufs=3))
            psa = ctx.enter_context(tc.tile_pool(name="psa", bufs=2,
                                                 space="PSUM"))
            psy = ctx.enter_context(tc.tile_pool(name="psy", bufs=2,
                                                 space="PSUM"))

            ws = cpool.tile([128, 256], FP, tag="w")
            nc.sync.dma_start(ws[:], wts[:])
            ysl = cpool.tile([128, 32], FP, tag="ysl")
            nc.sync.dma_start(ysl[:], ysel[:])
            b2s = cpool.tile([32, 128], FP, tag="b2")
            nc.sync.dma_start(b2s[:], bias2[:])
            idn = cpool.tile([32, 32], FP, tag="id")
            nc.sync.dma_start(idn[:], iden[:])

            u_prev = upool.tile([128, 32], FP, tag="u")
            nc.vector.memset(u_prev[:], 0.25)

            pre_tiles = [None] * ((t_steps + scan_ch - 1) // scan_ch)
            y_tile = None
            for rep, k in ((r, kk) for r in range(repeats)
                           for kk in range(t_steps + 2)):
                if k < t_steps and k % scan_ch == 0:
                    ci = k // scan_ch
                    w = min(scan_ch, t_steps - k) * 128
                    pt = ppool.tile([32, scan_ch * 128], FP, tag="p")
                    nc.sync.dma_start(pt[:, 0:w], pre0[:, k * 128:k * 128 + w])
                    pre_tiles[ci] = pt
                col = min(k, t_steps - 1)
                pt = pre_tiles[col // scan_ch]
                cc = (col % scan_ch) * 128

                pa = psa.tile([128, 64], FP, tag="pa")
                # pa layout: cols 0:32 = {s1 (rows 0:64), s2 (rows 64:128)},
                # cols 32:64 = {gneg (rows 0:64), g (rows 64:128)};
                # each 64-row half is [cells01(32); L2(32)] in unit order.
                # State P = [s1*gm; s2*g] (128,32); u = P_top + P_bot happens
                # inside the next matmul via K=128 row-replicated weights.
                nc.tensor.matmul(pa[0:32, :], idn[:], pt[:, cc:cc + 64],
                                 start=True, stop=False, tile_position=(0, 0))
                nc.tensor.matmul(pa[32:64, :], idn[:], b2s[:, 0:64],
                                 start=True, stop=False, tile_position=(0, 32))
                nc.tensor.matmul(pa[64:96, :], idn[:], pt[:, cc + 64:cc + 128],
                                 start=True, stop=False, tile_position=(0, 64))
                nc.tensor.matmul(pa[96:128, :], idn[:], b2s[:, 64:128],
                                 start=True, stop=False, tile_position=(0, 96))
                nc.tensor.matmul(pa[:, 0:32], ws[:, 0:128], u_prev[:],
                                 start=False, stop=False, tile_position=(0, 0))
                nc.tensor.matmul(pa[:, 32:64], ws[:, 128:256], u_prev[:],
                                 start=False, stop=True, tile_position=(0, 0))

                sa = spool.tile([128, 64], FP, tag="sa")
                nc.scalar.activation(sa[:], pa[:], AF.Sigmoid)

                u_new = upool.tile([128, 32], FP, tag="u")
                nc.vector.tensor_mul(u_new[:], sa[:, 0:32], sa[:, 32:64])
                if k == 0:
                    nc.vector.memset(u_new[0:12, :], 0.25)
                    nc.vector.memset(u_new[32:64, :], 0.25)
                    nc.vector.memset(u_new[64:76, :], 0.25)
                    nc.vector.memset(u_new[96:128, :], 0.25)
                elif k == 1:
                    nc.vector.memset(u_new[32:64, :], 0.25)
                    nc.vector.memset(u_new[96:128, :], 0.25)
                if k >= 2:
                    t_out = k - 2
                    if t_out % scan_ch == 0:
                        y_tile = ypool.tile([N2, scan_ch * B], FP, tag="y")
                    yc = (t_out % scan_ch) * B
                    yp = psy.tile([32, 32], FP, tag="yp")
                    nc.tensor.matmul(yp[:], ysl[:], u_new[:],
                                     start=True, stop=True,
                                     tile_position=(0, 0))
                    nc.vector.tensor_copy(y_tile[:, yc:yc + B], yp[:])
                    if t_out % scan_ch == scan_ch - 1 or t_out == t_steps - 1:
                        c0 = (t_out // scan_ch) * scan_ch * B
                        nc.sync.dma_start(
                            yout[:, c0:c0 + (t_out % scan_ch + 1) * B],
                            y_tile[:, 0:(t_out % scan_ch + 1) * B])
                u_prev = u_new
    nc.compile()
    return nc


# ----------------------------------------------------------------- launch C
def _build_launch_c():
    nc = bacc.Bacc("TRN2", target_bir_lowering=False, debug=False,
                   num_devices=NCORES)
    yf = nc.dram_tensor("yf", [N2, TOK], FP, kind="ExternalInput").ap()
    yb = nc.dram_tensor("yb", [N2, TOK], FP, kind="ExternalInput").ap()
    wc1 = nc.dram_tensor("wc1", [2 * N2, H], FP, kind="ExternalInput").ap()
    bc1 = nc.dram_tensor("bc1", [H // 2, 2], FP, kind="ExternalInput").ap()
    wc2 = nc.dram_tensor("wc2", [H // 2, 2 * H], FP, kind="ExternalInput").ap()
    wd1 = nc.dram_tensor("wd1", [H // 2, 256], FP, kind="ExternalInput").ap()
    bd1 = nc.dram_tensor("bd1", [128, 1], FP, kind="ExternalInput").ap()
    wd2 = nc.dram_tensor("wd2", [128, 64], FP, kind="ExternalInput").ap()
    bd2 = nc.dram_tensor("bd2", [64, 1], FP, kind="ExternalInput").ap()
    wd3 = nc.dram_tensor("wd3", [64, 2], FP, kind="ExternalInput").ap()
    bd3 = nc.dram_tensor("bd3", [2, 1], FP, kind="ExternalInput").ap()
    psi = nc.dram_tensor("psi", [2, TOK], FP, kind="ExternalOutput").ap()

    with tile.TileContext(nc) as tc:
        with ExitStack() as ctx:
            cpool = ctx.enter_context(tc.tile_pool(name="const", bufs=1))
            upool = ctx.enter_context(tc.tile_pool(name="u", bufs=2))
            hpool = ctx.enter_context(tc.tile_pool(name="h", bufs=2))
            opool = ctx.enter_context(tc.tile_pool(name="o", bufs=2))
            pspool = ctx.enter_context(tc.tile_pool(name="ps", bufs=2,
                                                    space="PSUM"))

            wc1s = cpool.tile([2 * N2, H], FP, tag="wc1")
            nc.sync.dma_start(wc1s[:], wc1[:])
            bc1s = cpool.tile([H // 2, 2], FP, tag="bc1")
            nc.sync.dma_start(bc1s[:], bc1[:])
            wc2s = cpool.tile([H // 2, 2 * H], FP, tag="wc2")
            nc.sync.dma_start(wc2s[:], wc2[:])
            wd1s = cpool.tile([H // 2, 256], FP, tag="wd1")
            nc.sync.dma_start(wd1s[:], wd1[:])
            bd1s = cpool.tile([128, 1], FP, tag="bd1")
            nc.sync.dma_start(bd1s[:], bd1[:])
            wd2s = cpool.tile([128, 64], FP, tag="wd2")
            nc.sync.dma_start(wd2s[:], wd2[:])
            bd2s = cpool.tile([64, 1], FP, tag="bd2")
            nc.sync.dma_start(bd2s[:], bd2[:])
            wd3s = cpool.tile([64, 2], FP, tag="wd3")
            nc.sync.dma_start(wd3s[:], wd3[:])
            bd3s = cpool.tile([2, 1], FP, tag="bd3")
            nc.sync.dma_start(bd3s[:], bd3[:])

            nt = CHUNK // 512
            for c0 in range(0, TOK, CHUNK):
                us = upool.tile([2 * N2, CHUNK], FP, tag="us")
                nc.sync.dma_start(us[0:N2, :], yf[:, c0:c0 + CHUNK])
                nc.sync.dma_start(us[N2:2 * N2, :], yb[:, c0:c0 + CHUNK])
                # comb layer 1 (K=64) + SiLU
                c1 = [hpool.tile([128, CHUNK], FP, tag=f"c1_{i}", name=f"c1_{i}_{c0}") for i in range(2)]
                for half in range(2):
                    for j in range(nt):
                        ps = pspool.tile([128, 512], FP, tag="ps")
                        nc.tensor.matmul(ps[:],
                                         wc1s[:, half * 128:(half + 1) * 128],
                                         us[:, j * 512:(j + 1) * 512],
                                         start=True, stop=True)
                        nc.scalar.activation(c1[half][:, j * 512:(j + 1) * 512],
                                             ps[:], AF.Silu,
                                             bias=bc1s[:, half:half + 1])
                # comb layer 2 (no act, bias folded into dec1)
                c2 = [hpool.tile([128, CHUNK], FP, tag=f"c2_{i}", name=f"c2_{i}_{c0}") for i in range(2)]
                for half in range(2):
                    for j in range(nt):
                        ps = pspool.tile([128, 512], FP, tag="ps")
                        for kh in range(2):
                            nc.tensor.matmul(
                                ps[:],
                                wc2s[:, kh * H + half * 128:kh * H + (half + 1) * 128],
                                c1[kh][:, j * 512:(j + 1) * 512],
                                start=(kh == 0), stop=(kh == 1))
                        nc.scalar.copy(c2[half][:, j * 512:(j + 1) * 512], ps[:])
                # dec layer 1 (256->128) + SiLU
                d1 = hpool.tile([128, CHUNK], FP, tag="d1")
                for j in range(nt):
                    ps = pspool.tile([128, 512], FP, tag="ps")
                    for kh in range(2):
                        nc.tensor.matmul(ps[:], wd1s[:, kh * 128:(kh + 1) * 128],
                                         c2[kh][:, j * 512:(j + 1) * 512],
                                         start=(kh == 0), stop=(kh == 1))
                    nc.scalar.activation(d1[:, j * 512:(j + 1) * 512], ps[:],
                                         AF.Silu, bias=bd1s[:])
                # dec layer 2 (128->64) + SiLU
                d2 = hpool.tile([64, CHUNK], FP, tag="d2")
                for j in range(nt):
                    ps = pspool.tile([64, 512], FP, tag="ps64")
                    nc.tensor.matmul(ps[:], wd2s[:], d1[:, j * 512:(j + 1) * 512],
                                     start=True, stop=True)
                    nc.scalar.activation(d2[:, j * 512:(j + 1) * 512], ps[:],
                                         AF.Silu, bias=bd2s[:])
                # dec layer 3 (64->2) + bias
                po = opool.tile([2, CHUNK], FP, tag="po")
                for j in range(nt):
                    ps = pspool.tile([2, 512], FP, tag="ps2")
                    nc.tensor.matmul(ps[:], wd3s[:], d2[:, j * 512:(j + 1) * 512],
                                     start=True, stop=True)
                    nc.vector.tensor_scalar_add(po[:, j * 512:(j + 1) * 512],
                                                ps[:], bd3s[:])
                nc.sync.dma_start(psi[:, c0:c0 + CHUNK], po[:])
    nc.compile()
    return nc


# ------------------------------------------------------------- host helpers
def _np(a):
    return np.asarray(a, dtype=np.float32)


def _pack2(w):
    """(256, C) -> (128, 2C): halves of the contraction dim side by side."""
    return np.ascontiguousarray(np.concatenate([w[:128], w[128:]], axis=1))


def _gate_parts(p, m, d):
    """Masked x/h weight parts + bias for each gate of one CfC cell."""
    out = {}
    m = _np(m)
    for gname in ("ff1", "ff2", "ta", "tb"):
        wm = _np(p[gname + "_w"]) * m
        out[gname] = (wm[:d], wm[d:], _np(p[gname + "_b"]))
    return out


def _scan_params(cfc, masks, b3):
    """Host-side weight transforms for one scan direction.

    sigmoid-space: state u = (h+1)/2, gates s = sigma(...).  A weight W
    consuming a (2u-1) input becomes 2W with bias correction -colsum(W);
    ff gates additionally get an overall factor 2 (tanh(x)=2*sig(2x)-1).

    Returns:
      px   (H, 96): pre0 projection weights.  Unit order inside each
                    32-row gate block is [L1-bias rows (12) | L0 units (20)]
      pb   (96, 1): pre0 bias; rows c:c+12 carry the L1 gate biases
      wts  (128, 96): recurrent weights; input rows [u1(12); u0(20)] at
                    0:32 and [u1copy(12); u0copy(20, zero); u2(32)] at 64:128
      bias2 (32, 96): L2 gate biases broadcast over batch
    """
    g0 = _gate_parts(cfc[0], masks[0], H)
    g1 = _gate_parts(cfc[1], masks[1], N0)
    g2 = _gate_parts(cfc[2], masks[2], N1)

    px = np.zeros((H, 128), np.float32)
    pb = np.zeros((128, 1), np.float32)
    gw = {}
    gb2 = {}
    for gn in ("ff1", "ff2", "g", "gneg"):
        if gn == "gneg":
            gw[gn] = -gw["g"]
            gb2[gn] = -gb2["g"]
            continue
        if gn == "g":
            w0x = g0["ta"][0] + g0["tb"][0]
            w0h = g0["ta"][1] + g0["tb"][1]
            b0v = g0["ta"][2] + g0["tb"][2]
            w1x = g1["ta"][0] + g1["tb"][0]
            w1h = g1["ta"][1] + g1["tb"][1]
            b1v = g1["ta"][2] + g1["tb"][2]
            w2x = g2["ta"][0] + g2["tb"][0]
            w2h = g2["ta"][1] + g2["tb"][1]
            b2v = g2["ta"][2] + g2["tb"][2]
            f = 1.0
        else:
            w0x, w0h, b0v = g0[gn]
            w1x, w1h, b1v = g1[gn]
            w2x, w2h, b2v = g2[gn]
            f = 2.0
        # per-gate weight block (64, 64): rows [u1(12); u0(20); u2(32)],
        # cols [cells01: L1(12) L0(20) | L2(32)]
        wg = np.zeros((64, 64), np.float32)
        wg[0:12, 0:12] = 2.0 * f * w1h
        wg[12:32, 0:12] = 2.0 * f * w1x
        wg[12:32, 12:32] = 2.0 * f * w0h
        wg[0:12, 32:64] = 2.0 * f * w2x
        wg[32:64, 32:64] = 2.0 * f * w2h
        gw[gn] = wg
        # pre0 x-proj cols + bias, unit order [L1-bias(12); L0(20)]
        pxg = np.zeros((H, 32), np.float32)
        pxg[:, 12:32] = f * w0x
        pbg = np.zeros(32, np.float32)
        pbg[0:12] = f * (b1v - w1x.sum(0) - w1h.sum(0))
        pbg[12:32] = f * (b0v + b3 @ w0x - w0h.sum(0))
        gw[gn + "_px"] = pxg
        gw[gn + "_pb"] = pbg
        gb2[gn] = f * (b2v - w2x.sum(0) - w2h.sum(0))
    if "gneg_px" not in gw:
        pass
    gw["gneg_px"] = -gw["g_px"]
    gw["gneg_pb"] = -gw["g_pb"]
    # pre0 DRAM block order per timestep: [s1 | gneg | s2 | g]
    order = ("ff1", "gneg", "ff2", "g")
    for i, gn in enumerate(order):
        px[:, i * 32:(i + 1) * 32] = gw[gn + "_px"]
        pb[i * 32:(i + 1) * 32, 0] = gw[gn + "_pb"]
    # wts (128, 256): col-block 0 = [s1-outs | s2-outs] (128 cols),
    # col-block 1 = [gneg-outs | g-outs]; rows = u rows replicated twice
    wts = np.zeros((128, 256), np.float32)
    for ci, (ga, gb_) in enumerate((("ff1", "ff2"), ("gneg", "g"))):
        blkc = np.concatenate([gw[ga], gw[gb_]], axis=1)     # (64, 128)
        wts[:, ci * 128:(ci + 1) * 128] = np.concatenate([blkc, blkc], axis=0)
    # bias2 (32, 128): [b2_s1 | b2_gneg | b2_s2 | b2_g] broadcast over batch
    bias2 = np.zeros((32, 128), np.float32)
    for i, gn in enumerate(("ff1", "gneg", "ff2", "g")):
        bias2[:, i * 32:(i + 1) * 32] = gb2[gn][:, None]
    ysel = np.zeros((128, 32), np.float32)
    ysel[32:64] = np.eye(32, dtype=np.float32)
    ysel[96:128] = np.eye(32, dtype=np.float32)
    return px, pb, wts, bias2, ysel


def _ident_tile():
    return np.eye(32, dtype=np.float32)


# ------------------------------------------------------------------- kernel
def kernel(x, enc, cfc_f, cfc_b, comb, dec, masks_f, masks_b):
    if "A" not in _prog_cache:
        _prog_cache["A"] = _build_launch_a()
        _prog_cache["B"] = _build_launch_b()
        _prog_cache["C"] = _build_launch_c()
    ncA, ncB, ncC = _prog_cache["A"], _prog_cache["B"], _prog_cache["C"]

    x = _np(x)
    (w1, b1), (w2, b2), (w3, b3) = [(_np(w), _np(b)) for w, b in enc]
    pxf, pbf, wtsf, bias2f, ysel = _scan_params(cfc_f, masks_f, b3)
    pxb, pbb, wtsb, bias2b, _ = _scan_params(cfc_b, masks_b, b3)
    iden = _ident_tile()

    # ---- launch A: encoder + pre0, time-sharded across 8 cores
    xT = np.ascontiguousarray(x.transpose(2, 1, 0).reshape(IN, T * B))
    common_a = dict(w1=w1, b1=_pack2(b1.reshape(H, 1)), w2=_pack2(w2),
                    b2=_pack2(b2.reshape(H, 1)), w3=_pack2(w3),
                    pxf=_pack2(pxf), pbf=pbf, pxb=_pack2(pxb), pbb=pbb)
    in_maps = [dict(common_a, xT=np.ascontiguousarray(
        xT[:, c * TOK:(c + 1) * TOK])) for c in range(NCORES)]
    res_a = run_bass_kernel_spmd(ncA, in_maps, core_ids=list(range(NCORES)))
    pre0f = np.concatenate([res_a.results[c]["pre0f"] for c in range(NCORES)],
                           axis=1)
    pre0b = np.concatenate([res_a.results[c]["pre0b"] for c in range(NCORES)],
                           axis=1)
    # bwd scan consumes time-reversed sequence
    pre0b = np.ascontiguousarray(
        pre0b.reshape(32, T, 128)[:, ::-1, :].reshape(32, T * 128))

    # ---- launch B: the two scans (core 0 fwd, core 1 bwd)
    in_b = [dict(pre0=pre0f, wts=wtsf, bias2=bias2f, ident=iden, ysel=ysel),
            dict(pre0=pre0b, wts=wtsb, bias2=bias2b, ident=iden, ysel=ysel)]
    res_b = run_bass_kernel_spmd(ncB, in_b, core_ids=[0, 1])
    y_f = res_b.results[0]["y"]
    y_b = res_b.results[1]["y"]
    y_b = np.ascontiguousarray(
        y_b.reshape(N2, T, B)[:, ::-1, :].reshape(N2, T * B))

    # ---- launch C: comb + dec
    (wc1, bc1), (wc2, bc2) = [(_np(w), _np(b)) for w, b in comb]
    (wd1, bd1), (wd2, bd2), (wd3, bd3) = [(_np(w), _np(b)) for w, b in dec]
    wc1t = 2.0 * wc1
    bc1t = bc1 - wc1.sum(0)
    bd1t = bd1 + bc2 @ wd1
    common_c = dict(wc1=wc1t, bc1=_pack2(bc1t.reshape(H, 1)), wc2=_pack2(wc2),
                    wd1=_pack2(wd1), bd1=bd1t.reshape(128, 1),
                    wd2=wd2, bd2=bd2.reshape(64, 1),
                    wd3=wd3, bd3=bd3.reshape(2, 1))
    in_maps = [dict(common_c,
                    yf=np.ascontiguousarray(y_f[:, c * TOK:(c + 1) * TOK]),
                    yb=np.ascontiguousarray(y_b[:, c * TOK:(c + 1) * TOK]))
               for c in range(NCORES)]
    res_c = run_bass_kernel_spmd(ncC, in_maps, core_ids=list(range(NCORES)))
    psi = np.concatenate([res_c.results[c]["psi"] for c in range(NCORES)],
                         axis=1)                     # (2, T*B) t-major
    psi = psi.reshape(2, T, B).transpose(0, 2, 1)    # (2, B, T)
    return np.ascontiguousarray(psi)


# revision 25
# speedup vs baseline: 2536.6928x; 2132.9750x over previous
"""Bidirectional CfC (AutoNCP-wired) Trainium2 Bass kernel.

Strategy
--------
The network is: encoder MLP (9->256->256->256, SiLU) -> two CfC scans
(fwd + bwd over T=2048, 3 stacked cells of sizes 20/12/32) -> comb MLP
(64->256->256) -> decoder MLP (256->128->64->2).

The MLP parts are embarrassingly parallel over B*T tokens and run
data-parallel over all 8 cores (sharded by time, transposed layout:
features on partitions, tokens on the free dim).  The CfC scans are
strictly sequential in T and latency-bound, so each direction runs on a
single core (core 0 fwd, core 1 bwd) with batch=32 on the free dim.

Cell math is rewritten in "sigmoid space": with tanh(x) = 2*sigmoid(2x)-1
and state u = (h+1)/2, every gate becomes a plain sigmoid and the affine
corrections fold into host-preprocessed weights/biases.  The three cells
run as a skewed pipeline (tick k computes h0(k), h1(k-1), h2(k-2)) so the
per-tick critical path is a single cell: matmul -> sigmoid -> 3 vector ops.

Three SPMD launches:
  A (8 cores): encoder + layer-0 input projections ("pre0", 60 gate rows)
  B (2 cores): the scans (same program, per-core data: fwd / time-flipped bwd)
  C (8 cores): comb + decoder on the scan outputs
"""

import os
import numpy as np
from contextlib import ExitStack

import concourse.bass as bass
import concourse.tile as tile
from concourse import bacc, mybir
from concourse.bass_utils import run_bass_kernel_spmd

FP = mybir.dt.float32
AF = mybir.ActivationFunctionType

B, T, IN, H = 32, 2048, 9, 256
N0, N1, N2 = 20, 12, 32          # CfC layer sizes (inter / cmd / motor)
NCORES = 8
TC = T // NCORES                 # timesteps per core in bulk launches
TOK = TC * B                     # tokens per core (t-major, b-minor)
CHUNK = 2048                     # token chunk in bulk launches
SCAN_CH = 128                    # timesteps per DMA chunk in the scan

_prog_cache = {}


# ----------------------------------------------------------------- launch A
def _build_launch_a():
    nc = bacc.Bacc("TRN2", target_bir_lowering=False, debug=False,
                   num_devices=NCORES)
    xT = nc.dram_tensor("xT", [IN, TOK], FP, kind="ExternalInput").ap()
    w1 = nc.dram_tensor("w1", [IN, H], FP, kind="ExternalInput").ap()
    b1 = nc.dram_tensor("b1", [H // 2, 2], FP, kind="ExternalInput").ap()
    w2 = nc.dram_tensor("w2", [H // 2, 2 * H], FP, kind="ExternalInput").ap()
    b2 = nc.dram_tensor("b2", [H // 2, 2], FP, kind="ExternalInput").ap()
    w3 = nc.dram_tensor("w3", [H // 2, 2 * H], FP, kind="ExternalInput").ap()
    pxf = nc.dram_tensor("pxf", [H // 2, 256], FP, kind="ExternalInput").ap()
    pbf = nc.dram_tensor("pbf", [128, 1], FP, kind="ExternalInput").ap()
    pxb = nc.dram_tensor("pxb", [H // 2, 256], FP, kind="ExternalInput").ap()
    pbb = nc.dram_tensor("pbb", [128, 1], FP, kind="ExternalInput").ap()
    of = nc.dram_tensor("pre0f", [32, TC * 128], FP, kind="ExternalOutput").ap()
    ob = nc.dram_tensor("pre0b", [32, TC * 128], FP, kind="ExternalOutput").ap()

    with tile.TileContext(nc) as tc:
        with ExitStack() as ctx:
            cpool = ctx.enter_context(tc.tile_pool(name="const", bufs=1))
            xpool = ctx.enter_context(tc.tile_pool(name="x", bufs=2))
            hpool = ctx.enter_context(tc.tile_pool(name="h", bufs=2))
            opool = ctx.enter_context(tc.tile_pool(name="o", bufs=2))
            pspool = ctx.enter_context(tc.tile_pool(name="ps", bufs=2,
                                                    space="PSUM"))

            # 256-row weights are stored in SBUF as (128, 2*cols):
            # w_sb[a, kh*cols + c] = w[kh*128 + a, c]
            w1s = cpool.tile([IN, H], FP, tag="w1")
            nc.sync.dma_start(w1s[:], w1[:])
            b1s = cpool.tile([H // 2, 2], FP, tag="b1")
            nc.sync.dma_start(b1s[:], b1[:])
            w2s = cpool.tile([H // 2, 2 * H], FP, tag="w2")
            nc.sync.dma_start(w2s[:], w2[:])
            b2s = cpool.tile([H // 2, 2], FP, tag="b2")
            nc.sync.dma_start(b2s[:], b2[:])
            w3s = cpool.tile([H // 2, 2 * H], FP, tag="w3")
            nc.sync.dma_start(w3s[:], w3[:])
            pxfs = cpool.tile([H // 2, 256], FP, tag="pxf")
            nc.sync.dma_start(pxfs[:], pxf[:])
            pbfs = cpool.tile([128, 1], FP, tag="pbf")
            nc.sync.dma_start(pbfs[:], pbf[:])
            pxbs = cpool.tile([H // 2, 256], FP, tag="pxb")
            nc.sync.dma_start(pxbs[:], pxb[:])
            pbbs = cpool.tile([128, 1], FP, tag="pbb")
            nc.sync.dma_start(pbbs[:], pbb[:])

            nt = CHUNK // 512
            for c0 in range(0, TOK, CHUNK):
                xs = xpool.tile([IN, CHUNK], FP, tag="xs")
                nc.sync.dma_start(xs[:], xT[:, c0:c0 + CHUNK])

                # ---- enc layer 1 (K=9) + SiLU
                h1 = [hpool.tile([128, CHUNK], FP, tag=f"h1_{i}", name=f"h1_{i}_{c0}") for i in range(2)]
                for half in range(2):
                    for j in range(nt):
                        ps = pspool.tile([128, 512], FP, tag="ps")
                        nc.tensor.matmul(ps[:], w1s[:, half * 128:(half + 1) * 128],
                                         xs[:, j * 512:(j + 1) * 512],
                                         start=True, stop=True)
                        nc.scalar.activation(h1[half][:, j * 512:(j + 1) * 512],
                                             ps[:], AF.Silu,
                                             bias=b1s[:, half:half + 1])
                # ---- enc layer 2 + SiLU
                h2 = [hpool.tile([128, CHUNK], FP, tag=f"h2_{i}", name=f"h2_{i}_{c0}") for i in range(2)]
                for half in range(2):
                    for j in range(nt):
                        ps = pspool.tile([128, 512], FP, tag="ps")
                        for kh in range(2):
                            nc.tensor.matmul(
                                ps[:],
                                w2s[:, kh * H + half * 128:kh * H + (half + 1) * 128],
                                h1[kh][:, j * 512:(j + 1) * 512],
                                start=(kh == 0), stop=(kh == 1))
                        nc.scalar.activation(h2[half][:, j * 512:(j + 1) * 512],
                                             ps[:], AF.Silu,
                                             bias=b2s[:, half:half + 1])
                # ---- enc layer 3 (no activation; bias folded into pre0)
                h3 = [hpool.tile([128, CHUNK], FP, tag=f"h3_{i}", name=f"h3_{i}_{c0}") for i in range(2)]
                for half in range(2):
                    for j in range(nt):
                        ps = pspool.tile([128, 512], FP, tag="ps")
                        for kh in range(2):
                            nc.tensor.matmul(
                                ps[:],
                                w3s[:, kh * H + half * 128:kh * H + (half + 1) * 128],
                                h2[kh][:, j * 512:(j + 1) * 512],
                                start=(kh == 0), stop=(kh == 1))
                        nc.scalar.copy(h3[half][:, j * 512:(j + 1) * 512], ps[:])
                # ---- pre0 projections (fwd & bwd)
                # po rows: [s1(20)+b1row(12) | s2(20)+b1row(12) | g(20)+b1row(12)]
                # DRAM pre0 layout: (32, t*96): col = t*96 + gate*32 + b
                for pxs, pbs, out in ((pxfs, pbfs, of), (pxbs, pbbs, ob)):
                    po = opool.tile([128, CHUNK], FP, tag="po")
                    for j in range(nt):
                        ps = pspool.tile([128, 512], FP, tag="ps96")
                        for kh in range(2):
                            nc.tensor.matmul(
                                ps[:],
                                pxs[:, kh * 128:(kh + 1) * 128],
                                h3[kh][:, j * 512:(j + 1) * 512],
                                start=(kh == 0), stop=(kh == 1))
                        nc.vector.tensor_scalar_add(
                            po[:, j * 512:(j + 1) * 512], ps[:], pbs[:])
                    t0, t1 = c0 // B, (c0 + CHUNK) // B
                    dst = out.rearrange("r (t x) -> r t x", x=128)
                    for gi in range(4):
                        nc.sync.dma_start(
                            dst[:, t0:t1, gi * 32:(gi + 1) * 32],
                            po[gi * 32:(gi + 1) * 32, :].rearrange(
                                "r (t b) -> r t b", b=B))
    nc.compile()
    return nc


# ----------------------------------------------------------------- launch B
def _build_launch_b(t_steps=T, scan_ch=SCAN_CH, repeats=1):
    nc = bacc.Bacc("TRN2", target_bir_lowering=False, debug=False,
                   num_devices=2)
    pre0 = nc.dram_tensor("pre0", [32, t_steps * 128], FP,
                          kind="ExternalInput").ap()
    wts = nc.dram_tensor("wts", [128, 256], FP, kind="ExternalInput").ap()
    ysel = nc.dram_tensor("ysel", [128, 32], FP, kind="ExternalInput").ap()
    bias2 = nc.dram_tensor("bias2", [32, 128], FP, kind="ExternalInput").ap()
    iden = nc.dram_tensor("ident", [32, 32], FP, kind="ExternalInput").ap()
    yout = nc.dram_tensor("y", [N2, t_steps * B], FP,
                          kind="ExternalOutput").ap()

    with tile.TileContext(nc) as tc:
        with ExitStack() as ctx:
            cpool = ctx.enter_context(tc.tile_pool(name="const", bufs=1))
            ppool = ctx.enter_context(tc.tile_pool(name="pre0", bufs=2))
            ypool = ctx.enter_context(tc.tile_pool(name="y", bufs=2))
            upool = ctx.enter_context(tc.tile_pool(name="u", bufs=3))
            spool = ctx.enter_context(tc.tile_pool(name="s", bufs=3))
            psa = ctx.enter_context(tc.tile_pool(name="psa", bufs=2,
                                                 space="PSUM"))
            psy = ctx.enter_context(tc.tile_pool(name="psy", bufs=2,
                                                 space="PSUM"))

            ws = cpool.tile([128, 256], FP, tag="w")
            nc.sync.dma_start(ws[:], wts[:])
            ysl = cpool.tile([128, 32], FP, tag="ysl")
            nc.sync.dma_start(ysl[:], ysel[:])
            b2s = cpool.tile([32, 128], FP, tag="b2")
            nc.sync.dma_start(b2s[:], bias2[:])
            idn = cpool.tile([32, 32], FP, tag="id")
            nc.sync.dma_start(idn[:], iden[:])

            u_prev = upool.tile([128, 32], FP, tag="u")
            nc.vector.memset(u_prev[:], 0.25)

            pre_tiles = [None] * ((t_steps + scan_ch - 1) // scan_ch)
            y_tile = None
            for rep, k in ((r, kk) for r in range(repeats)
                           for kk in range(t_steps + 2)):
                if k < t_steps and k % scan_ch == 0:
                    ci = k // scan_ch
                    w = min(scan_ch, t_steps - k) * 128
                    pt = ppool.tile([32, scan_ch * 128], FP, tag="p")
                    nc.sync.dma_start(pt[:, 0:w], pre0[:, k * 128:k * 128 + w])
                    pre_tiles[ci] = pt
                col = min(k, t_steps - 1)
                pt = pre_tiles[col // scan_ch]
                cc = (col % scan_ch) * 128

                pa = psa.tile([128, 64], FP, tag="pa")
                # pa layout: cols 0:32 = {s1 (rows 0:64), s2 (rows 64:128)},
                # cols 32:64 = {gneg (rows 0:64), g (rows 64:128)};
                # each 64-row half is [cells01(32); L2(32)] in unit order.
                # State P = [s1*gm; s2*g] (128,32); u = P_top + P_bot happens
                # inside the next matmul via K=128 row-replicated weights.
                nc.tensor.matmul(pa[0:32, :], idn[:], pt[:, cc:cc + 64],
                                 start=True, stop=False, tile_position=(0, 0))
                nc.tensor.matmul(pa[32:64, :], idn[:], b2s[:, 0:64],
                                 start=True, stop=False, tile_position=(0, 32))
                nc.tensor.matmul(pa[64:96, :], idn[:], pt[:, cc + 64:cc + 128],
                                 start=True, stop=False, tile_position=(0, 64))
                nc.tensor.matmul(pa[96:128, :], idn[:], b2s[:, 64:128],
                                 start=True, stop=False, tile_position=(0, 96))
                nc.tensor.matmul(pa[:, 0:32], ws[:, 0:128], u_prev[:],
                                 start=False, stop=False, tile_position=(0, 0))
                nc.tensor.matmul(pa[:, 32:64], ws[:, 128:256], u_prev[:],
                                 start=False, stop=True, tile_position=(0, 0))

                sa = spool.tile([128, 64], FP, tag="sa")
                nc.scalar.activation(sa[:], pa[:], AF.Sigmoid)

                u_new = upool.tile([128, 32], FP, tag="u")
                nc.vector.tensor_mul(u_new[:], sa[:, 0:32], sa[:, 32:64])
                if k == 0:
                    nc.vector.memset(u_new[0:12, :], 0.25)
                    nc.vector.memset(u_new[32:64, :], 0.25)
                    nc.vector.memset(u_new[64:76, :], 0.25)
                    nc.vector.memset(u_new[96:128, :], 0.25)
                elif k == 1:
                    nc.vector.memset(u_new[32:64, :], 0.25)
                    nc.vector.memset(u_new[96:128, :], 0.25)
                if k >= 2:
                    t_out = k - 2
                    if t_out % scan_ch == 0:
                        y_tile = ypool.tile([N2, scan_ch * B], FP, tag="y")
                    yc = (t_out % scan_ch) * B
                    yp = psy.tile([32, 32], FP, tag="yp")
                    nc.tensor.matmul(yp[:], ysl[:], u_new[:],
                                     start=True, stop=True,
                                     tile_position=(0, 0))
                    nc.vector.tensor_copy(y_tile[:, yc:yc + B], yp[:])
                    if t_out % scan_ch == scan_ch - 1 or t_out == t_steps - 1:
                        c0 = (t_out // scan_ch) * scan_ch * B
                        nc.sync.dma_start(
                            yout[:, c0:c0 + (t_out % scan_ch + 1) * B],
                            y_tile[:, 0:(t_out % scan_ch + 1) * B])
                u_prev = u_new
    nc.compile()
    return nc


# ----------------------------------------------------------------- launch C
def _build_launch_c():
    nc = bacc.Bacc("TRN2", target_bir_lowering=False, debug=False,
                   num_devices=NCORES)
    yf = nc.dram_tensor("yf", [N2, TOK], FP, kind="ExternalInput").ap()
    yb = nc.dram_tensor("yb", [N2, TOK], FP, kind="ExternalInput").ap()
    wc1 = nc.dram_tensor("wc1", [2 * N2, H], FP, kind="ExternalInput").ap()
    bc1 = nc.dram_tensor("bc1", [H // 2, 2], FP, kind="ExternalInput").ap()
    wc2 = nc.dram_tensor("wc2", [H // 2, 2 * H], FP, kind="ExternalInput").ap()
    wd1 = nc.dram_tensor("wd1", [H // 2, 256], FP, kind="ExternalInput").ap()
    bd1 = nc.dram_tensor("bd1", [128, 1], FP, kind="ExternalInput").ap()
    wd2 = nc.dram_tensor("wd2", [128, 64], FP, kind="ExternalInput").ap()
    bd2 = nc.dram_tensor("bd2", [64, 1], FP, kind="ExternalInput").ap()
    wd3 = nc.dram_tensor("wd3", [64, 2], FP, kind="ExternalInput").ap()
    bd3 = nc.dram_tensor("bd3", [2, 1], FP, kind="ExternalInput").ap()
    psi = nc.dram_tensor("psi", [2, TOK], FP, kind="ExternalOutput").ap()

    with tile.TileContext(nc) as tc:
        with ExitStack() as ctx:
            cpool = ctx.enter_context(tc.tile_pool(name="const", bufs=1))
            upool = ctx.enter_context(tc.tile_pool(name="u", bufs=2))
            hpool = ctx.enter_context(tc.tile_pool(name="h", bufs=2))
            opool = ctx.enter_context(tc.tile_pool(name="o", bufs=2))
            pspool = ctx.enter_context(tc.tile_pool(name="ps", bufs=2,
                                                    space="PSUM"))

            wc1s = cpool.tile([2 * N2, H], FP, tag="wc1")
            nc.sync.dma_start(wc1s[:], wc1[:])
            bc1s = cpool.tile([H // 2, 2], FP, tag="bc1")
            nc.sync.dma_start(bc1s[:], bc1[:])
            wc2s = cpool.tile([H // 2, 2 * H], FP, tag="wc2")
            nc.sync.dma_start(wc2s[:], wc2[:])
            wd1s = cpool.tile([H // 2, 256], FP, tag="wd1")
            nc.sync.dma_start(wd1s[:], wd1[:])
            bd1s = cpool.tile([128, 1], FP, tag="bd1")
            nc.sync.dma_start(bd1s[:], bd1[:])
            wd2s = cpool.tile([128, 64], FP, tag="wd2")
            nc.sync.dma_start(wd2s[:], wd2[:])
            bd2s = cpool.tile([64, 1], FP, tag="bd2")
            nc.sync.dma_start(bd2s[:], bd2[:])
            wd3s = cpool.tile([64, 2], FP, tag="wd3")
            nc.sync.dma_start(wd3s[:], wd3[:])
            bd3s = cpool.tile([2, 1], FP, tag="bd3")
            nc.sync.dma_start(bd3s[:], bd3[:])

            nt = CHUNK // 512
            for c0 in range(0, TOK, CHUNK):
                us = upool.tile([2 * N2, CHUNK], FP, tag="us")
                nc.sync.dma_start(us[0:N2, :], yf[:, c0:c0 + CHUNK])
                nc.sync.dma_start(us[N2:2 * N2, :], yb[:, c0:c0 + CHUNK])
                # comb layer 1 (K=64) + SiLU
                c1 = [hpool.tile([128, CHUNK], FP, tag=f"c1_{i}", name=f"c1_{i}_{c0}") for i in range(2)]
                for half in range(2):
                    for j in range(nt):
                        ps = pspool.tile([128, 512], FP, tag="ps")
                        nc.tensor.matmul(ps[:],
                                         wc1s[:, half * 128:(half + 1) * 128],
                                         us[:, j * 512:(j + 1) * 512],
                                         start=True, stop=True)
                        nc.scalar.activation(c1[half][:, j * 512:(j + 1) * 512],
                                             ps[:], AF.Silu,
                                             bias=bc1s[:, half:half + 1])
                # comb layer 2 (no act, bias folded into dec1)
                c2 = [hpool.tile([128, CHUNK], FP, tag=f"c2_{i}", name=f"c2_{i}_{c0}") for i in range(2)]
                for half in range(2):
                    for j in range(nt):
                        ps = pspool.tile([128, 512], FP, tag="ps")
                        for kh in range(2):
                            nc.tensor.matmul(
                                ps[:],
                                wc2s[:, kh * H + half * 128:kh * H + (half + 1) * 128],
                                c1[kh][:, j * 512:(j + 1) * 512],
                                start=(kh == 0), stop=(kh == 1))
                        nc.scalar.copy(c2[half][:, j * 512:(j + 1) * 512], ps[:])
                # dec layer 1 (256->128) + SiLU
                d1 = hpool.tile([128, CHUNK], FP, tag="d1")
                for j in range(nt):
                    ps = pspool.tile([128, 512], FP, tag="ps")
                    for kh in range(2):
                        nc.tensor.matmul(ps[:], wd1s[:, kh * 128:(kh + 1) * 128],
                                         c2[kh][:, j * 512:(j + 1) * 512],
                                         start=(kh == 0), stop=(kh == 1))
                    nc.scalar.activation(d1[:, j * 512:(j + 1) * 512], ps[:],
                                         AF.Silu, bias=bd1s[:])
                # dec layer 2 (128->64) + SiLU
                d2 = hpool.tile([64, CHUNK], FP, tag="d2")
                for j in range(nt):
                    ps = pspool.tile([64, 512], FP, tag="ps64")
                    nc.tensor.matmul(ps[:], wd2s[:], d1[:, j * 512:(j + 1) * 512],
                                     start=True, stop=True)
                    nc.scalar.activation(d2[:, j * 512:(j + 1) * 512], ps[:],
                                         AF.Silu, bias=bd2s[:])
                # dec layer 3 (64->2) + bias
                po = opool.tile([2, CHUNK], FP, tag="po")
                for j in range(nt):
                    ps = pspool.tile([2, 512], FP, tag="ps2")
                    nc.tensor.matmul(ps[:], wd3s[:], d2[:, j * 512:(j + 1) * 512],
                                     start=True, stop=True)
                    nc.vector.tensor_scalar_add(po[:, j * 512:(j + 1) * 512],
                                                ps[:], bd3s[:])
                nc.sync.dma_start(psi[:, c0:c0 + CHUNK], po[:])
    nc.compile()
    return nc


# ------------------------------------------------------------- host helpers
def _np(a):
    return np.asarray(a, dtype=np.float32)


def _pack2(w):
    """(256, C) -> (128, 2C): halves of the contraction dim side by side."""
    return np.ascontiguousarray(np.concatenate([w[:128], w[128:]], axis=1))


def _gate_parts(p, m, d):
    """Masked x/h weight parts + bias for each gate of one CfC cell."""
    out = {}
    m = _np(m)
    for gname in ("ff1", "ff2", "ta", "tb"):
        wm = _np(p[gname + "_w"]) * m
        out[gname] = (wm[:d], wm[d:], _np(p[gname + "_b"]))
    return out


def _scan_params(cfc, masks, b3):
    """Host-side weight transforms for one scan direction.

    sigmoid-space: state u = (h+1)/2, gates s = sigma(...).  A weight W
    consuming a (2u-1) input becomes 2W with bias correction -colsum(W);
    ff gates additionally get an overall factor 2 (tanh(x)=2*sig(2x)-1).

    Returns:
      px   (H, 96): pre0 projection weights.  Unit order inside each
                    32-row gate block is [L1-bias rows (12) | L0 units (20)]
      pb   (96, 1): pre0 bias; rows c:c+12 carry the L1 gate biases
      wts  (128, 96): recurrent weights; input rows [u1(12); u0(20)] at
                    0:32 and [u1copy(12); u0copy(20, zero); u2(32)] at 64:128
      bias2 (32, 96): L2 gate biases broadcast over batch
    """
    g0 = _gate_parts(cfc[0], masks[0], H)
    g1 = _gate_parts(cfc[1], masks[1], N0)
    g2 = _gate_parts(cfc[2], masks[2], N1)

    px = np.zeros((H, 128), np.float32)
    pb = np.zeros((128, 1), np.float32)
    gw = {}
    gb2 = {}
    for gn in ("ff1", "ff2", "g", "gneg"):
        if gn == "gneg":
            gw[gn] = -gw["g"]
            gb2[gn] = -gb2["g"]
            continue
        if gn == "g":
            w0x = g0["ta"][0] + g0["tb"][0]
            w0h = g0["ta"][1] + g0["tb"][1]
            b0v = g0["ta"][2] + g0["tb"][2]
            w1x = g1["ta"][0] + g1["tb"][0]
            w1h = g1["ta"][1] + g1["tb"][1]
            b1v = g1["ta"][2] + g1["tb"][2]
            w2x = g2["ta"][0] + g2["tb"][0]
            w2h = g2["ta"][1] + g2["tb"][1]
            b2v = g2["ta"][2] + g2["tb"][2]
            f = 1.0
        else:
            w0x, w0h, b0v = g0[gn]
            w1x, w1h, b1v = g1[gn]
            w2x, w2h, b2v = g2[gn]
            f = 2.0
        # per-gate weight block (64, 64): rows [u1(12); u0(20); u2(32)],
        # cols [cells01: L1(12) L0(20) | L2(32)]
        wg = np.zeros((64, 64), np.float32)
        wg[0:12, 0:12] = 2.0 * f * w1h
        wg[12:32, 0:12] = 2.0 * f * w1x
        wg[12:32, 12:32] = 2.0 * f * w0h
        wg[0:12, 32:64] = 2.0 * f * w2x
        wg[32:64, 32:64] = 2.0 * f * w2h
        gw[gn] = wg
        # pre0 x-proj cols + bias, unit order [L1-bias(12); L0(20)]
        pxg = np.zeros((H, 32), np.float32)
        pxg[:, 12:32] = f * w0x
        pbg = np.zeros(32, np.float32)
        pbg[0:12] = f * (b1v - w1x.sum(0) - w1h.sum(0))
        pbg[12:32] = f * (b0v + b3 @ w0x - w0h.sum(0))
        gw[gn + "_px"] = pxg
        gw[gn + "_pb"] = pbg
        gb2[gn] = f * (b2v - w2x.sum(0) - w2h.sum(0))
    if "gneg_px" not in gw:
        pass
    gw["gneg_px"] = -gw["g_px"]
    gw["gneg_pb"] = -gw["g_pb"]
    # pre0 DRAM block order per timestep: [s1 | gneg | s2 | g]
    order = ("ff1", "gneg", "ff2", "g")
    for i, gn in enumerate(order):
        px[:, i * 32:(i + 1) * 32] = gw[gn + "_px"]
        pb[i * 32:(i + 1) * 32, 0] = gw[gn + "_pb"]
    # wts (128, 256): col-block 0 = [s1-outs | s2-outs] (128 cols),
    # col-block 1 = [gneg-outs | g-outs]; rows = u rows replicated twice
    wts = np.zeros((128, 256), np.float32)
    for ci, (ga, gb_) in enumerate((("ff1", "ff2"), ("gneg", "g"))):
        blkc = np.concatenate([gw[ga], gw[gb_]], axis=1)     # (64, 128)
        wts[:, ci * 128:(ci + 1) * 128] = np.concatenate([blkc, blkc], axis=0)
    # bias2 (32, 128): [b2_s1 | b2_gneg | b2_s2 | b2_g] broadcast over batch
    bias2 = np.zeros((32, 128), np.float32)
    for i, gn in enumerate(("ff1", "gneg", "ff2", "g")):
        bias2[:, i * 32:(i + 1) * 32] = gb2[gn][:, None]
    ysel = np.zeros((128, 32), np.float32)
    ysel[32:64] = np.eye(32, dtype=np.float32)
    ysel[96:128] = np.eye(32, dtype=np.float32)
    return px, pb, wts, bias2, ysel


def _ident_tile():
    return np.eye(32, dtype=np.float32)


# ------------------------------------------------------------------- kernel
def kernel(x, enc, cfc_f, cfc_b, comb, dec, masks_f, masks_b):
    if "A" not in _prog_cache:
        _prog_cache["A"] = _build_launch_a()
        _prog_cache["B"] = _build_launch_b()
        _prog_cache["C"] = _build_launch_c()
    ncA, ncB, ncC = _prog_cache["A"], _prog_cache["B"], _prog_cache["C"]

    x = _np(x)
    (w1, b1), (w2, b2), (w3, b3) = [(_np(w), _np(b)) for w, b in enc]
    pxf, pbf, wtsf, bias2f, ysel = _scan_params(cfc_f, masks_f, b3)
    pxb, pbb, wtsb, bias2b, _ = _scan_params(cfc_b, masks_b, b3)
    iden = _ident_tile()

    # ---- launch A: encoder + pre0, time-sharded across 8 cores
    xT = np.ascontiguousarray(x.transpose(2, 1, 0).reshape(IN, T * B))
    common_a = dict(w1=w1, b1=_pack2(b1.reshape(H, 1)), w2=_pack2(w2),
                    b2=_pack2(b2.reshape(H, 1)), w3=_pack2(w3),
                    pxf=_pack2(pxf), pbf=pbf, pxb=_pack2(pxb), pbb=pbb)
    in_maps = [dict(common_a, xT=np.ascontiguousarray(
        xT[:, c * TOK:(c + 1) * TOK])) for c in range(NCORES)]
    res_a = run_bass_kernel_spmd(ncA, in_maps, core_ids=list(range(NCORES)))
    pre0f = np.concatenate([res_a.results[c]["pre0f"] for c in range(NCORES)],
                           axis=1)
    pre0b = np.concatenate([res_a.results[c]["pre0b"] for c in range(NCORES)],
                           axis=1)
    # bwd scan consumes time-reversed sequence
    pre0b = np.ascontiguousarray(
        pre0b.reshape(32, T, 128)[:, ::-1, :].reshape(32, T * 128))

    # ---- launch B: the two scans (core 0 fwd, core 1 bwd)
    in_b = [dict(pre0=pre0f, wts=wtsf, bias2=bias2f, ident=iden, ysel=ysel),
            dict(pre0=pre0b, wts=wtsb, bias2=bias2b, ident=iden, ysel=ysel)]
    res_b = run_bass_kernel_spmd(ncB, in_b, core_ids=[0, 1])
    y_f = res_b.results[0]["y"]
    y_b = res_b.results[1]["y"]
    y_b = np.ascontiguousarray(
        y_b.reshape(N2, T, B)[:, ::-1, :].reshape(N2, T * B))

    # ---- launch C: comb + dec
    (wc1, bc1), (wc2, bc2) = [(_np(w), _np(b)) for w, b in comb]
    (wd1, bd1), (wd2, bd2), (wd3, bd3) = [(_np(w), _np(b)) for w, b in dec]
    wc1t = 2.0 * wc1
    bc1t = bc1 - wc1.sum(0)
    bd1t = bd1 + bc2 @ wd1
    common_c = dict(wc1=wc1t, bc1=_pack2(bc1t.reshape(H, 1)), wc2=_pack2(wc2),
                    wd1=_pack2(wd1), bd1=bd1t.reshape(128, 1),
                    wd2=wd2, bd2=bd2.reshape(64, 1),
                    wd3=wd3, bd3=bd3.reshape(2, 1))
    in_maps = [dict(common_c,
                    yf=np.ascontiguousarray(y_f[:, c * TOK:(c + 1) * TOK]),
                    yb=np.ascontiguousarray(y_b[:, c * TOK:(c + 1) * TOK]))
               for c in range(NCORES)]
    res_c = run_bass_kernel_spmd(ncC, in_maps, core_ids=list(range(NCORES)))
    psi = np.concatenate([res_c.results[c]["psi"] for c in range(NCORES)],
                         axis=1)                     # (2, T*B) t-major
    psi = psi.reshape(2, T, B).transpose(0, 2, 1)    # (2, B, T)
    return np.ascontiguousarray(psi)


def estimate_hw_time_ns():
    """Predicted on-device time per kernel() call: sum of the three
    launches' single-core TimelineSim totals (SPMD; cores run identical
    programs, launches are serialized)."""
    from concourse.timeline_sim import TimelineSim
    if "A" not in _prog_cache:
        _prog_cache["A"] = _build_launch_a()
        _prog_cache["B"] = _build_launch_b()
        _prog_cache["C"] = _build_launch_c()
    tot = 0.0
    for k in ("A", "B", "C"):
        tot += TimelineSim(_prog_cache[k]).simulate()
    return tot
